# revision 20
# baseline (speedup 1.0000x reference)
"""BTT (block tensor-train) structured FC kernel for Trainium2, 8-core data parallel.

Math: y[b, (oa ob oc od)] = sum_blk sum_{r*} F0[ia,oa,ra] F1[ib,ob,rb] F2[ic,oc,rc]
F3[id,od,rd] C[rd,rc,rb,ra] x[b, (ia ib ic id)]  with all mode dims 8, ranks 2.

Host folds factors into:
  G[icid, blk, q=(rc,rd), ocod] = F2[ic,oc,rc]*F3[id,od,rd]          (stage A rhs)
  W[blk, q, iaib, oaob] = sum_{ra,rb} C[rd,rc,rb,ra] F0[ia,oa,ra] F1[ib,ob,rb]
Sharding is pure batch data-parallel (128 rows per core); as part of sharding
the host lays tensors out in the on-chip format.

Both stages run as full-array K=128 matmuls with block-diagonal packing of the
two batch parities (zeros in the off-diagonal blocks), which meets the
out-lane x logical-K throughput bound without array tiling:
  stage A: lhsT = [p=(blockdiag icid x b_lo), m=(b_lo, iaib)] per batch pair,
           rhs = G duplicated on both partition halves, out u[(b_lo,iaib), .]
  stage B: lhsT = blockdiag W per chunk k, rhs = u, accumulate 16 chunks,
           out y[(b_lo, oaob), (bp, ocod)]
DTYPE bf16 (default) or f32r (accurate fallback, ~2x slower PE, uses the
older 2-row-stream structure since f32r forbids column tiling).
"""

import os

import numpy as np

N_CORES = 8
B_CORE = 128
DTYPE = os.environ.get("BTT_DTYPE", "bf16")

_CACHE = {}


def _fold_weights_base(cores, factors):
    cores = np.asarray(cores, dtype=np.float64)      # (4, 2,2,2,2) [rd,rc,rb,ra]
    factors = np.asarray(factors, dtype=np.float64)  # (4, 4, 8, 8, 2)
    G = np.zeros((64, 4, 4, 64), np.float64)         # [icid, blk, q, ocod]
    W = np.zeros((4, 4, 64, 64), np.float64)         # [blk, q, iaib, oaob]
    for blk in range(4):
        F0, F1, F2, F3 = (factors[blk, j] for j in range(4))
        C = cores[blk]
        G[:, blk] = np.einsum("cxr,dys->cdrsxy", F2, F3).reshape(64, 4, 64)
        w = np.einsum("srqp,axp,byq->srabxy", C, F0, F1).transpose(1, 0, 2, 3, 4, 5)
        W[blk] = w.reshape(4, 64, 64)
    g2 = G.reshape(64, 1024)                               # [icid, (blk q ocod)]
    w3 = W.reshape(16, 64, 64)                             # [k, iaib, oaob]
    return g2, w3


def _fold_weights(cores, factors):
    g2, w3 = _fold_weights_base(cores, factors)
    g_dup = np.concatenate([g2, g2], axis=0)               # [128, 1024]
    if DTYPE == "bf16":
        # block-diagonal W: [p, k, b_lo, oaob]
        w_bd = np.zeros((128, 16, 2, 64), np.float64)
        for k in range(16):
            w_bd[0:64, k, 0, :] = w3[k]
            w_bd[64:128, k, 1, :] = w3[k]
        import ml_dtypes
        return (g_dup.astype(ml_dtypes.bfloat16),
                np.ascontiguousarray(w_bd.reshape(128, 2048)).astype(
                    ml_dtypes.bfloat16))
    w2 = np.ascontiguousarray(w3.transpose(1, 0, 2).reshape(64, 1024))
    w_dup = np.concatenate([w2, w2], axis=0)               # [128, 1024]
    return g_dup.astype(np.float32), w_dup.astype(np.float32)


def _build_nc():
    import concourse.mybir as mybir
    from concourse import bacc
    from concourse.tile import TileContext

    f32 = mybir.dt.float32
    bf16 = mybir.dt.bfloat16
    f32r = mybir.dt.float32r
    dt_op = bf16 if DTYPE == "bf16" else f32r
    dt_w = bf16 if DTYPE == "bf16" else f32

    nc = bacc.Bacc("TRN2", target_bir_lowering=False, debug=False,
                   num_devices=N_CORES)
    if DTYPE == "bf16":
        # xt: host-prepared block-diagonal input [p, bp, b_lo, iaib]
        xt_d = nc.dram_tensor("xt", [128, 64, 2, 64], dt_w, kind="ExternalInput")
        w_d = nc.dram_tensor("w", [128, 2048], dt_w, kind="ExternalInput")
    else:
        xt_d = nc.dram_tensor("xt", [64, 128, 64], dt_w, kind="ExternalInput")
        w_d = nc.dram_tensor("w", [128, 1024], dt_w, kind="ExternalInput")
    g_d = nc.dram_tensor("g", [128, 1024], dt_w, kind="ExternalInput")
    # y: partition-major [p=(b_lo, oaob), (pair, h, bp, ocod)]
    y_d = nc.dram_tensor("y", [128, 4096], f32, kind="ExternalOutput")

    with TileContext(nc) as tc:
        with tc.tile_pool(name="const", bufs=1) as const, \
             tc.tile_pool(name="upool", bufs=2) as upool:

            g_sb = const.tile([128, 1024], dt_op, tag="g_sb")
            if DTYPE == "bf16":
                w_sb = const.tile([128, 16, 2, 64], dt_op, tag="w_sb")
                nc.sync.dma_start(g_sb[:], g_d[:])
                nc.sync.dma_start(w_sb[:], w_d[:].rearrange(
                    "p (k bl m) -> p k bl m", k=16, bl=2))
                xz = const.tile([128, 64, 2, 64], bf16, tag="xz")
                for j in range(4):
                    nc.sync.dma_start(xz[:, j * 16:(j + 1) * 16, :, :],
                                      xt_d[:, j * 16:(j + 1) * 16, :, :])
            else:
                w_sb = const.tile([128, 1024], dt_op, tag="w_sb")
                g_raw = const.tile([128, 1024], f32, tag="g_raw")
                w_raw = const.tile([128, 1024], f32, tag="w_raw")
                nc.sync.dma_start(g_raw[:], g_d[:])
                nc.sync.dma_start(w_raw[:], w_d[:])
                nc.vector.tensor_copy(g_sb[:], g_raw[:])
                nc.scalar.copy(w_sb[:], w_raw[:])
                xraw = const.tile([128, 128, 64], f32, tag="xraw")
                nc.sync.dma_start(xraw[0:64, :, :], xt_d[:])
                nc.sync.dma_start(xraw[64:128, :, :], xt_d[:])
                xT2 = const.tile([128, 128, 64], f32r, tag="xT2")
                nc.vector.tensor_copy(xT2[0:64], xraw[0:64])
                nc.scalar.copy(xT2[64:128], xraw[64:128])

            # y_sb[p=(b_lo, oaob), pair, h, bp, ocod]
            y_sb = const.tile([128, 4, 2, 8, 64], f32, tag="y_sb")

            a_bufs, b_bufs = (4, 4) if DTYPE == "bf16" else (4, 4)
            with tc.tile_pool(name="apsum", bufs=a_bufs, space="PSUM") as apsum, \
                 tc.tile_pool(name="bpsum", bufs=b_bufs, space="PSUM") as bpsum:
                for pair in range(4):
                    # u holds two bgroups (16 batch pairs)
                    u = upool.tile([128, 16, 1024], dt_op, tag="u")
                    for p2 in range(8):
                        bpe = pair * 16 + 2 * p2
                        bpo = bpe + 1
                        ps_el = apsum.tile([128, 512], f32, tag="aps")
                        ps_eh = apsum.tile([128, 512], f32, tag="aps")
                        ps_ol = apsum.tile([128, 512], f32, tag="aps")
                        ps_oh = apsum.tile([128, 512], f32, tag="aps")
                        if DTYPE == "bf16":
                            for bp, pl, ph in ((bpe, ps_el, ps_eh),
                                               (bpo, ps_ol, ps_oh)):
                                lhs = xz[:, bp, :, :]
                                nc.tensor.matmul(pl[:], lhs, g_sb[:, 0:512],
                                                 start=True, stop=True)
                                nc.tensor.matmul(ph[:], lhs, g_sb[:, 512:1024],
                                                 start=True, stop=True)
                        else:
                            lhs_e = xT2[0:64, 2 * bpe:2 * bpe + 2, :]
                            lhs_o = xT2[64:128, 2 * bpo:2 * bpo + 2, :]
                            nc.tensor.matmul(ps_el[:], lhs_e,
                                             g_sb[0:64, 0:512],
                                             start=True, stop=True)
                            nc.tensor.matmul(ps_ol[:], lhs_o,
                                             g_sb[64:128, 0:512],
                                             start=True, stop=True)
                            nc.tensor.matmul(ps_eh[:], lhs_e,
                                             g_sb[0:64, 512:1024],
                                             start=True, stop=True)
                            nc.tensor.matmul(ps_oh[:], lhs_o,
                                             g_sb[64:128, 512:1024],
                                             start=True, stop=True)
                        nc.scalar.copy(u[:, 2 * p2, 0:512], ps_el[:])
                        nc.vector.tensor_copy(u[:, 2 * p2, 512:1024], ps_eh[:])
                        nc.vector.tensor_copy(u[:, 2 * p2 + 1, 0:512], ps_ol[:])
                        nc.scalar.copy(u[:, 2 * p2 + 1, 512:1024], ps_oh[:])

                    if DTYPE == "bf16":
                        # stage B: blockdiag W, one K=128 N=512 matmul per
                        # (k, h); h = bgroup of the pair.
                        psb0 = bpsum.tile([128, 512], f32, tag="bps")
                        psb1 = bpsum.tile([128, 512], f32, tag="bps")
                        for k in range(16):
                            st = (k == 0)
                            sp = (k == 15)
                            for h, psb in ((0, psb0), (1, psb1)):
                                nc.tensor.matmul(
                                    psb[:],
                                    w_sb[:, k, :, :],
                                    u[:, h * 8:(h + 1) * 8, k * 64:(k + 1) * 64],
                                    start=st, stop=sp)
                        for h, psb in ((0, psb0), (1, psb1)):
                            dst = y_sb[:, pair, h, :, :]
                            if h == 0:
                                nc.scalar.copy(dst, psb[:])
                            else:
                                nc.vector.tensor_copy(dst, psb[:])
                    else:
                        psb0 = bpsum.tile([128, 512], f32, tag="bps")
                        psb1 = bpsum.tile([128, 512], f32, tag="bps")
                        psb2 = bpsum.tile([128, 512], f32, tag="bps")
                        psb3 = bpsum.tile([128, 512], f32, tag="bps")
                        for k in range(16):
                            st = (k == 0)
                            sp = (k == 15)
                            for r in (0, 1):
                                for h in range(2):
                                    psb = (psb0, psb1, psb2, psb3)[r * 2 + h]
                                    nc.tensor.matmul(
                                        psb[0:64, :],
                                        w_sb[r * 64:(r + 1) * 64,
                                             k * 64:(k + 1) * 64],
                                        u[r * 64:(r + 1) * 64, h * 8:(h + 1) * 8,
                                          k * 64:(k + 1) * 64],
                                        start=st, stop=sp,
                                        tile_position=(r * 64, 0),
                                    )
                        # f32r y layout: p=(b_lo r, oaob) needs both r psums
                        for r in (0, 1):
                            for h in range(2):
                                psb = (psb0, psb1, psb2, psb3)[r * 2 + h]
                                dst = y_sb[r * 64:(r + 1) * 64, pair, h, :, :]
                                src = psb[0:64, :]
                                if (r + h) % 2 == 0:
                                    nc.scalar.copy(dst, src)
                                else:
                                    nc.vector.tensor_copy(dst, src)

                    nc.sync.dma_start(
                        y_d[:, pair * 1024:(pair + 1) * 1024],
                        y_sb[:, pair, :, :, :])

    nc.compile()
    return nc


def kernel(inputs, cores, factors, trace=False):
    x = np.ascontiguousarray(np.asarray(inputs, dtype=np.float32))
    assert x.shape == (N_CORES * B_CORE, 4096), x.shape
    g_dup, w_host = _fold_weights(cores, factors)

    from concourse.bass_utils import run_bass_kernel_spmd

    if "nc" not in _CACHE:
        _CACHE["nc"] = _build_nc()
    nc = _CACHE["nc"]

    in_maps = []
    for c in range(N_CORES):
        xc = x[c * B_CORE:(c + 1) * B_CORE].reshape(128, 64, 64)  # [b, iaib, icid]
        if DTYPE == "bf16":
            import ml_dtypes
            xz = np.zeros((128, 64, 2, 64), np.float32)
            xz[0:64, :, 0, :] = xc[0::2].transpose(2, 0, 1)   # [icid, bp, iaib]
            xz[64:128, :, 1, :] = xc[1::2].transpose(2, 0, 1)
            xt = xz.astype(ml_dtypes.bfloat16)
        else:
            xt = np.ascontiguousarray(xc.transpose(2, 0, 1)).astype(np.float32)
        in_maps.append({"xt": xt, "g": g_dup, "w": w_host})

    res = run_bass_kernel_spmd(nc, in_maps, core_ids=list(range(N_CORES)),
                               trace=trace)
    _CACHE["last_result"] = res

    out = np.empty((N_CORES * B_CORE, 4096), np.float32)
    for c in range(N_CORES):
        yp = res.results[c]["y"]                       # [128, 4096]
        yr = yp.reshape(2, 64, 4, 2, 8, 64)            # [b_lo, oaob, pair, h, bp, oc]
        yb = yr.transpose(2, 3, 4, 0, 1, 5).reshape(128, 4096)
        out[c * B_CORE:(c + 1) * B_CORE] = yb
    return out
